# revision 2
# baseline (speedup 1.0000x reference)
"""Trainium2 Bass kernel for nn_AdaptiveEpisodicMemory (scatter_memory).

Computes, for B=4096 queries over an M=65536-slot memory bank:

    scores = q @ K^T + 0.5 * c @ CTX^T + 0.3*exp(-0.1*(1-t))  (masked by used_slots)
    out    = softmax(scores) @ V

Strategy (8 NeuronCores):
  * Unused slots receive -1e9 scores; their softmax weight is exactly 0 in
    fp32, so the host drops them up-front (exact transformation) and pads the
    survivors to a multiple of 8*128. Shapes are chosen per-input at build
    time, so the kernel is correct for any input.
  * The memory bank (keys/contexts/values) is sharded across the 8 cores;
    query/context are replicated. Per core:
        S^T[m, b]  = KC_shard^T.T @ QC^T      (one K=96 matmul, float32r)
        P^T[m, b]  = exp(S^T + bias_m)        (ScalarE; bias folds the
                                               time-decay term and the mask)
        O^T[65, b] += Vaug_tile.T @ P^T       (Vaug = [V | 1]; row 64 of O^T
                                               accumulates the softmax denom)
  * ReduceScatter sums the [65, 4096] partials and hands each core its own
    512-query slice, which it transposes, divides by the denominator, and
    writes out. Host work is limited to layout: compaction/sharding/concat/
    transpose of inputs and concatenation of the 8 output slices.
"""
import sys

sys.path.insert(0, "/opt/trn_rl_repo")
import math

import numpy as np

from concourse import bass, bass_utils, masks, mybir, tile

B, M, D, CD = 4096, 65536, 64, 32
KDIM = D + CD  # 96: contraction dim of the fused score matmul
NCORES = 8
BCHUNK = 512
NBCHUNK = B // BCHUNK
F32 = mybir.dt.float32
F32R = mybir.dt.float32r
TIME_WEIGHT = 0.1
CURRENT_TIME = 1.0
DECAY_COEF = 0.3
NEG_BIG = -1e9


def _round_f32r(x: np.ndarray) -> np.ndarray:
    """Round-to-nearest fp32 -> fp32r (11-bit mantissa), as the PE expects."""
    u = np.ascontiguousarray(x, dtype=np.float32).view(np.uint32)
    r = (u + np.uint32(0x7FF) + ((u >> np.uint32(12)) & np.uint32(1))) & np.uint32(
        0xFFFFF000
    )
    return r.view(np.float32)


def _split_multi_waits(nc) -> int:
    """This walrus build accepts at most one fused sync-wait per instruction;
    hoist extras into standalone InstEventSemaphore instructions."""
    n_split = 0
    for fn in nc.m.functions:
        for bb in fn.blocks:
            insts = list(bb.instructions)
            out = []
            changed = False
            for inst in insts:
                si = inst.sync_info
                if si is not None and si.on_wait is not None and len(si.on_wait) > 1:
                    waits = list(si.on_wait)
                    for w in waits[:-1]:
                        ev = mybir.InstEventSemaphore(
                            name=f"{inst.name}-wsplit{n_split}",
                            engine=inst.engine,
                            ins=[],
                            outs=[],
                            sync_info=mybir.SyncInfo(on_wait=[w], on_update=[]),
                            bass_nofuse=True,
                        )
                        out.append(ev)
                        n_split += 1
                    inst.sync_info = mybir.SyncInfo(
                        on_wait=[waits[-1]], on_update=list(si.on_update or [])
                    )
                    changed = True
                out.append(inst)
            if changed:
                bb.instructions[:] = out
    return n_split


def _build(m_loc: int):
    """Build the per-core Bass program for a shard of m_loc memory slots."""
    ntiles = m_loc // 128
    nc = bass.Bass(trn_type="TRN2", debug=False, num_devices=NCORES)

    # register the decay-exp bias as a const AP (only 0.0/1.0 are built in)
    decay_bias = math.log(DECAY_COEF) - TIME_WEIGHT * CURRENT_TIME
    ct = nc.alloc_sbuf_tensor(f"const-float32-extra", [128, 1], F32)
    nc.gpsimd.memset(ct.ap(), decay_bias)
    nc.const_aps.aps[(F32, decay_bias)] = ct.ap()
    nc.all_engine_barrier()

    qc_ext = nc.dram_tensor("qc_t", [KDIM, B], F32R, kind="ExternalInput")
    kc_ext = nc.dram_tensor("kc_t", [KDIM, m_loc], F32R, kind="ExternalInput")
    va_ext = nc.dram_tensor("vaug", [m_loc, D + 1], F32R, kind="ExternalInput")
    ts_ext = nc.dram_tensor("tsm", [128, ntiles], F32, kind="ExternalInput")
    mk_ext = nc.dram_tensor("maskf", [128, ntiles], F32, kind="ExternalInput")
    out_ext = nc.dram_tensor("out", [BCHUNK, D], F32, kind="ExternalOutput")

    bounce = nc.dram_tensor("rs_in", [NCORES, D + 1, BCHUNK], F32)
    red = nc.dram_tensor("rs_out", [D + 1, BCHUNK], F32)

    with tile.TileContext(nc) as tc:
        with (
            tc.tile_pool(name="big", bufs=1) as big,
            tc.tile_pool(name="small", bufs=1) as small,
            tc.tile_pool(name="pT", bufs=4) as pTp,
            tc.tile_pool(name="osb", bufs=1) as osb,
            tc.tile_pool(name="psS", bufs=3, space="PSUM") as psS,
            tc.tile_pool(name="psO", bufs=2, space="PSUM") as psO,
            tc.tile_pool(name="psT", bufs=2, space="PSUM") as psT,
            tc.tile_pool(name="fin", bufs=2) as fin,
        ):
            qc_s = big.tile([KDIM, B], F32R)
            nc.sync.dma_start(qc_s[:], qc_ext.ap())
            kc_s = big.tile([KDIM, m_loc], F32R)
            nc.sync.dma_start(kc_s[:], kc_ext.ap())
            va_s = big.tile([128, ntiles * (D + 1)], F32R)
            nc.sync.dma_start(
                va_s[:].rearrange("p (k f) -> p k f", k=ntiles),
                va_ext.ap().rearrange("(k p) f -> p k f", p=128),
            )
            ts_s = small.tile([128, ntiles], F32)
            nc.sync.dma_start(ts_s[:], ts_ext.ap())
            mk_s = small.tile([128, ntiles], F32)
            nc.sync.dma_start(mk_s[:], mk_ext.ap())

            # identity for the epilogue transposes (GpSimd, overlaps DMAs)
            ident = small.tile([128, 128], F32)
            masks.make_identity(nc, ident[:])

            # bias_m = 0.3*exp(0.1*t - 0.1) + (mask ? 0 : -1e9)
            #        = exp(0.1*t + (ln 0.3 - 0.1)) + (mask - 1) * 1e9
            d_s = small.tile([128, ntiles], F32)
            nc.scalar.activation(
                d_s[:],
                ts_s[:],
                mybir.ActivationFunctionType.Exp,
                bias=math.log(DECAY_COEF) - TIME_WEIGHT * CURRENT_TIME,
                scale=TIME_WEIGHT,
            )
            mneg = small.tile([128, ntiles], F32)
            nc.vector.tensor_scalar(
                mneg[:],
                mk_s[:],
                -NEG_BIG,
                NEG_BIG,
                mybir.AluOpType.mult,
                mybir.AluOpType.add,
            )
            bias_s = small.tile([128, ntiles], F32)
            nc.vector.tensor_add(bias_s[:], d_s[:], mneg[:])

            oall = osb.tile([D + 1, B], F32)

            for c in range(NBCHUNK):
                oacc = psO.tile([D + 1, BCHUNK], F32)
                for k in range(ntiles):
                    sps = psS.tile([128, BCHUNK], F32)
                    nc.tensor.matmul(
                        sps[:],
                        lhsT=kc_s[:, 128 * k : 128 * (k + 1)],
                        rhs=qc_s[:, BCHUNK * c : BCHUNK * (c + 1)],
                        start=True,
                        stop=True,
                    )
                    pT = pTp.tile([128, BCHUNK], F32R)
                    nc.scalar.activation(
                        pT[:],
                        sps[:],
                        mybir.ActivationFunctionType.Exp,
                        bias=bias_s[:, k : k + 1],
                        scale=1.0,
                    )
                    nc.tensor.matmul(
                        oacc[:],
                        lhsT=va_s[:, (D + 1) * k : (D + 1) * (k + 1)],
                        rhs=pT[:],
                        start=(k == 0),
                        stop=(k == ntiles - 1),
                    )
                nc.vector.tensor_copy(oall[:, BCHUNK * c : BCHUNK * (c + 1)], oacc[:])

            # partials -> DRAM laid out so a flat ReduceScatter hands core i
            # the summed [65, 512] block for batch rows [512i, 512(i+1))
            nc.sync.dma_start(
                bounce.ap().rearrange("s p b -> p s b"),
                oall[:].rearrange("p (s b) -> p s b", s=NCORES),
            )
            nc.gpsimd.collective_compute(
                "ReduceScatter",
                mybir.AluOpType.add,
                replica_groups=[list(range(NCORES))],
                ins=[bounce.ap().opt()],
                outs=[red.ap().opt()],
            )
            r_s = fin.tile([D + 1, BCHUNK], F32, tag="rsred")
            nc.sync.dma_start(r_s[:], red.ap())

            for j in range(BCHUNK // 128):
                tp = psT.tile([128, D + 1], F32)
                nc.tensor.transpose(
                    tp[:],
                    in_=r_s[:, 128 * j : 128 * (j + 1)],
                    identity=ident[0 : D + 1, 0 : D + 1],
                )
                rcp = fin.tile([128, 1], F32, tag="rcp")
                nc.vector.reciprocal(rcp[:], tp[:, D : D + 1])
                ot = fin.tile([128, D], F32, tag="ot")
                nc.scalar.activation(
                    ot[:],
                    tp[:, 0:D],
                    mybir.ActivationFunctionType.Copy,
                    bias=0.0,
                    scale=rcp[:],
                )
                nc.sync.dma_start(out_ext.ap()[128 * j : 128 * (j + 1), :], ot[:])

    _split_multi_waits(nc)
    return nc


_BUILD_CACHE: dict[int, object] = {}


def kernel(
    query,
    context,
    mem_keys,
    mem_values,
    mem_contexts,
    mem_timestamps,
    used_slots,
    _want_trace: bool = False,
):
    query = np.asarray(query, dtype=np.float32)
    context = np.asarray(context, dtype=np.float32)
    mem_keys = np.asarray(mem_keys, dtype=np.float32)
    mem_values = np.asarray(mem_values, dtype=np.float32)
    mem_contexts = np.asarray(mem_contexts, dtype=np.float32)
    mem_timestamps = np.asarray(mem_timestamps, dtype=np.float32)
    used_slots = np.asarray(used_slots).astype(bool)

    idx = np.flatnonzero(used_slots)
    count = idx.size
    if count == 0:
        # softmax over uniformly -1e9 scores is uniform over all M slots
        return np.broadcast_to(
            mem_values.mean(axis=0, dtype=np.float64).astype(np.float32), (B, D)
        ).copy()

    m_loc = max(128, int(math.ceil(count / (NCORES * 128))) * 128)
    m_tot = m_loc * NCORES
    ntiles = m_loc // 128

    # host-side layout prep: compact used slots, pad, shard, fuse operands
    kc = np.zeros((m_tot, KDIM), dtype=np.float32)
    kc[:count, :D] = mem_keys[idx]
    kc[:count, D:] = mem_contexts[idx]
    va = np.zeros((m_tot, D + 1), dtype=np.float32)
    va[:count, :D] = mem_values[idx]
    va[:, D] = 1.0
    ts = np.zeros(m_tot, dtype=np.float32)
    ts[:count] = mem_timestamps[idx]
    mk = np.zeros(m_tot, dtype=np.float32)
    mk[:count] = 1.0

    qc = np.empty((B, KDIM), dtype=np.float32)
    qc[:, :D] = query
    qc[:, D:] = 0.5 * context
    qc_t = _round_f32r(qc.T)

    in_maps = []
    for s in range(NCORES):
        lo, hi = s * m_loc, (s + 1) * m_loc
        in_maps.append(
            {
                "qc_t": qc_t,
                "kc_t": _round_f32r(kc[lo:hi].T),
                "vaug": _round_f32r(va[lo:hi]),
                "tsm": np.ascontiguousarray(ts[lo:hi].reshape(ntiles, 128).T),
                "maskf": np.ascontiguousarray(mk[lo:hi].reshape(ntiles, 128).T),
            }
        )

    nc = _BUILD_CACHE.get(m_loc)
    if nc is None:
        nc = _build(m_loc)
        _BUILD_CACHE[m_loc] = nc

    res = bass_utils.run_bass_kernel_spmd(
        nc, in_maps, core_ids=list(range(NCORES)), trace=_want_trace
    )

    out = np.empty((B, D), dtype=np.float32)
    for s in range(NCORES):
        out[s * BCHUNK : (s + 1) * BCHUNK] = res.results[s]["out"]
    if _want_trace:
        kernel.last_exec_time_ns = res.exec_time_ns
        kernel.last_results = res
    return out


# revision 6
# speedup vs baseline: 1.0474x; 1.0474x over previous
"""Trainium2 Bass kernel for nn_AdaptiveEpisodicMemory (scatter_memory).

Computes, for B=4096 queries over an M=65536-slot memory bank:

    scores = q @ K^T + 0.5 * c @ CTX^T + 0.3*exp(-0.1*(1-t))  (masked by used_slots)
    out    = softmax(scores) @ V

Strategy (8 NeuronCores):
  * Unused slots receive -1e9 scores; their softmax weight is exactly 0 in
    fp32, so the host drops them up-front (exact transformation) and pads the
    survivors to a multiple of 8*128. Shapes are chosen per-input at build
    time, so the kernel is correct for any input.
  * The memory bank (keys/contexts/values) is sharded across the 8 cores;
    query/context are replicated. Per core:
        S^T[m, b]  = KC_shard^T.T @ QC^T      (one K=96 matmul, bf16)
        P^T[m, b]  = exp(S^T + bias_m)        (ScalarE; bias folds the
                                               time-decay term and the mask)
        O^T[65, b] += Vaug_tile.T @ P^T       (Vaug = [V | 1]; row 64 of O^T
                                               accumulates the softmax denom)
  * ReduceScatter sums the [65, 4096] partials and hands each core its own
    512-query slice, which it transposes, divides by the denominator, and
    writes out. Host work is limited to layout: compaction/sharding/concat/
    transpose of inputs and concatenation of the 8 output slices.
"""
import sys

sys.path.insert(0, "/opt/trn_rl_repo")
import math

import ml_dtypes
import numpy as np

from concourse import bass, bass_utils, masks, mybir, tile

B, M, D, CD = 4096, 65536, 64, 32
KDIM = D + CD  # 96: contraction dim of the fused score matmul
NCORES = 8
BCHUNK = 512
CPP = 2  # batch chunks per pass (exp runs at FD = CPP*BCHUNK)
NPASS = B // (BCHUNK * CPP)
F32 = mybir.dt.float32
BF16 = mybir.dt.bfloat16
TIME_WEIGHT = 0.1
CURRENT_TIME = 1.0
DECAY_COEF = 0.3
NEG_BIG = -1e9
N_WARMUP_MM = 26


def _split_multi_waits(nc) -> int:
    """This walrus build accepts at most one fused sync-wait per instruction;
    hoist extras into standalone InstEventSemaphore instructions."""
    n_split = 0
    for fn in nc.m.functions:
        for bb in fn.blocks:
            insts = list(bb.instructions)
            out = []
            changed = False
            for inst in insts:
                si = inst.sync_info
                if si is not None and si.on_wait is not None and len(si.on_wait) > 1:
                    waits = list(si.on_wait)
                    for w in waits[:-1]:
                        ev = mybir.InstEventSemaphore(
                            name=f"{inst.name}-wsplit{n_split}",
                            engine=inst.engine,
                            ins=[],
                            outs=[],
                            sync_info=mybir.SyncInfo(on_wait=[w], on_update=[]),
                            bass_nofuse=True,
                        )
                        out.append(ev)
                        n_split += 1
                    inst.sync_info = mybir.SyncInfo(
                        on_wait=[waits[-1]], on_update=list(si.on_update or [])
                    )
                    changed = True
                out.append(inst)
            if changed:
                bb.instructions[:] = out
    return n_split


def _build(m_loc: int):
    """Build the per-core Bass program for a shard of m_loc memory slots."""
    ntiles = m_loc // 128
    nc = bass.Bass(trn_type="TRN2", debug=False, num_devices=NCORES)

    # register the decay-exp bias as a const AP (only 0.0/1.0 are built in)
    decay_bias = math.log(DECAY_COEF) - TIME_WEIGHT * CURRENT_TIME
    ct = nc.alloc_sbuf_tensor("const-float32-extra", [128, 1], F32)
    nc.gpsimd.memset(ct.ap(), decay_bias)
    nc.const_aps.aps[(F32, decay_bias)] = ct.ap()
    nc.all_engine_barrier()

    qc_ext = nc.dram_tensor("qc_t", [KDIM, B], BF16, kind="ExternalInput")
    kc_ext = nc.dram_tensor("kc_t", [KDIM, m_loc], BF16, kind="ExternalInput")
    va_ext = nc.dram_tensor("vaug", [m_loc, D + 1], BF16, kind="ExternalInput")
    ts_ext = nc.dram_tensor("tsm", [128, ntiles], F32, kind="ExternalInput")
    mk_ext = nc.dram_tensor("maskf", [128, ntiles], F32, kind="ExternalInput")
    out_ext = nc.dram_tensor("out", [BCHUNK, D], F32, kind="ExternalOutput")

    bounce = nc.dram_tensor("rs_in", [NCORES, D + 1, BCHUNK], F32)
    red = nc.dram_tensor("rs_out", [D + 1, BCHUNK], F32)

    with tile.TileContext(nc) as tc:
        with (
            tc.tile_pool(name="big", bufs=1) as big,
            tc.tile_pool(name="small", bufs=1) as small,
            tc.tile_pool(name="pT", bufs=3) as pTp,
            tc.tile_pool(name="osb", bufs=1) as osb,
            tc.tile_pool(name="psS", bufs=2, space="PSUM") as psS,
            tc.tile_pool(name="psO", bufs=2, space="PSUM") as psO,
            tc.tile_pool(name="fin", bufs=2) as fin,
        ):
            # PE warmup: keep TensorE busy from t=0 so HAM reaches 2.4 GHz
            # before the real matmuls start (inputs are still DMAing in).
            wsrc = small.tile([128, 512], BF16)
            nc.vector.memset(wsrc[:], 1.0)
            wps = psS.tile([128, 512], F32, name="wps", tag="sps", padded_shape=[128, CPP * BCHUNK])
            for _ in range(N_WARMUP_MM):
                nc.tensor.matmul(
                    wps[:], lhsT=wsrc[:, 0:128], rhs=wsrc[:], start=True, stop=True
                )

            # input DMAs, chunked so consumers can start on the first pieces
            qc_s = big.tile([KDIM, B], BF16)
            for c in range(NPASS):
                w = B // NPASS
                nc.sync.dma_start(
                    qc_s[:, c * w : (c + 1) * w], qc_ext.ap()[:, c * w : (c + 1) * w]
                )
            kc_s = big.tile([KDIM, m_loc], BF16)
            for c in range(4):
                w = m_loc // 4
                nc.sync.dma_start(
                    kc_s[:, c * w : (c + 1) * w], kc_ext.ap()[:, c * w : (c + 1) * w]
                )
            va_s = big.tile([128, ntiles * (D + 1)], BF16)
            nc.sync.dma_start(
                va_s[:].rearrange("p (k f) -> p k f", k=ntiles),
                va_ext.ap().rearrange("(k p) f -> p k f", p=128),
            )
            ts_s = small.tile([128, ntiles], F32)
            nc.sync.dma_start(ts_s[:], ts_ext.ap())
            mk_s = small.tile([128, ntiles], F32)
            nc.sync.dma_start(mk_s[:], mk_ext.ap())

            # identity for the epilogue transposes (GpSimd, overlaps DMAs)
            ident = small.tile([128, 128], F32)
            masks.make_identity(nc, ident[:])

            # bias_m = 0.3*exp(0.1*t - 0.1) + (mask ? 0 : -1e9)
            #        = exp(0.1*t + (ln 0.3 - 0.1)) + (mask - 1) * 1e9
            d_s = small.tile([128, ntiles], F32)
            nc.scalar.activation(
                d_s[:],
                ts_s[:],
                mybir.ActivationFunctionType.Exp,
                bias=decay_bias,
                scale=TIME_WEIGHT,
            )
            mneg = small.tile([128, ntiles], F32)
            nc.vector.tensor_scalar(
                mneg[:],
                mk_s[:],
                -NEG_BIG,
                NEG_BIG,
                mybir.AluOpType.mult,
                mybir.AluOpType.add,
            )
            bias_s = small.tile([128, ntiles], F32)
            nc.vector.tensor_add(bias_s[:], d_s[:], mneg[:])

            oall = osb.tile([D + 1, B], F32)

            for p in range(NPASS):
                oaccs = [
                    psO.tile([D + 1, BCHUNK], F32, name=f"oacc{i}", tag=f"oacc{i}")
                    for i in range(CPP)
                ]
                for k in range(ntiles):
                    sps = psS.tile([128, CPP * BCHUNK], F32)
                    for i in range(CPP):
                        c = p * CPP + i
                        nc.tensor.matmul(
                            sps[:, i * BCHUNK : (i + 1) * BCHUNK],
                            lhsT=kc_s[:, 128 * k : 128 * (k + 1)],
                            rhs=qc_s[:, BCHUNK * c : BCHUNK * (c + 1)],
                            start=True,
                            stop=True,
                        )
                    pT = pTp.tile([128, CPP * BCHUNK], BF16)
                    nc.scalar.activation(
                        pT[:],
                        sps[:],
                        mybir.ActivationFunctionType.Exp,
                        bias=bias_s[:, k : k + 1],
                        scale=1.0,
                    )
                    for i in range(CPP):
                        nc.tensor.matmul(
                            oaccs[i][:],
                            lhsT=va_s[:, (D + 1) * k : (D + 1) * (k + 1)],
                            rhs=pT[:, i * BCHUNK : (i + 1) * BCHUNK],
                            start=(k == 0),
                            stop=(k == ntiles - 1),
                        )
                for i in range(CPP):
                    c = p * CPP + i
                    nc.vector.tensor_copy(
                        oall[:, BCHUNK * c : BCHUNK * (c + 1)], oaccs[i][:]
                    )

            # partials -> DRAM laid out so a flat ReduceScatter hands core i
            # the summed [65, 512] block for batch rows [512i, 512(i+1))
            nc.sync.dma_start(
                bounce.ap().rearrange("s p b -> p s b"),
                oall[:].rearrange("p (s b) -> p s b", s=NCORES),
            )
            nc.gpsimd.collective_compute(
                "ReduceScatter",
                mybir.AluOpType.add,
                replica_groups=[list(range(NCORES))],
                ins=[bounce.ap().opt()],
                outs=[red.ap().opt()],
            )
            r_s = fin.tile([D + 1, BCHUNK], F32, tag="rsred")
            nc.sync.dma_start(r_s[:], red.ap())

            for j in range(BCHUNK // 128):
                tp = psS.tile([128, D + 1], F32, tag="sps")
                nc.tensor.transpose(
                    tp[:],
                    in_=r_s[:, 128 * j : 128 * (j + 1)],
                    identity=ident[0 : D + 1, 0 : D + 1],
                )
                rcp = fin.tile([128, 1], F32, tag="rcp")
                nc.vector.reciprocal(rcp[:], tp[:, D : D + 1])
                ot = fin.tile([128, D], F32, tag="ot")
                nc.scalar.activation(
                    ot[:],
                    tp[:, 0:D],
                    mybir.ActivationFunctionType.Copy,
                    bias=0.0,
                    scale=rcp[:],
                )
                nc.sync.dma_start(out_ext.ap()[128 * j : 128 * (j + 1), :], ot[:])

    _split_multi_waits(nc)
    return nc


_BUILD_CACHE: dict[int, object] = {}


def kernel(
    query,
    context,
    mem_keys,
    mem_values,
    mem_contexts,
    mem_timestamps,
    used_slots,
    _want_trace: bool = False,
):
    query = np.asarray(query, dtype=np.float32)
    context = np.asarray(context, dtype=np.float32)
    mem_keys = np.asarray(mem_keys, dtype=np.float32)
    mem_values = np.asarray(mem_values, dtype=np.float32)
    mem_contexts = np.asarray(mem_contexts, dtype=np.float32)
    mem_timestamps = np.asarray(mem_timestamps, dtype=np.float32)
    used_slots = np.asarray(used_slots).astype(bool)

    idx = np.flatnonzero(used_slots)
    count = idx.size
    if count == 0:
        # softmax over uniformly -1e9 scores is uniform over all M slots
        return np.broadcast_to(
            mem_values.mean(axis=0, dtype=np.float64).astype(np.float32), (B, D)
        ).copy()

    m_loc = max(128, int(math.ceil(count / (NCORES * 128))) * 128)
    m_tot = m_loc * NCORES
    ntiles = m_loc // 128

    # host-side layout prep: compact used slots, pad, shard, fuse operands
    kc = np.zeros((m_tot, KDIM), dtype=np.float32)
    kc[:count, :D] = mem_keys[idx]
    kc[:count, D:] = mem_contexts[idx]
    va = np.zeros((m_tot, D + 1), dtype=np.float32)
    va[:count, :D] = mem_values[idx]
    va[:, D] = 1.0
    ts = np.zeros(m_tot, dtype=np.float32)
    ts[:count] = mem_timestamps[idx]
    mk = np.zeros(m_tot, dtype=np.float32)
    mk[:count] = 1.0

    qc = np.empty((B, KDIM), dtype=np.float32)
    qc[:, :D] = query
    qc[:, D:] = 0.5 * context
    qc_t = np.ascontiguousarray(qc.T).astype(ml_dtypes.bfloat16)

    in_maps = []
    for s in range(NCORES):
        lo, hi = s * m_loc, (s + 1) * m_loc
        in_maps.append(
            {
                "qc_t": qc_t,
                "kc_t": np.ascontiguousarray(kc[lo:hi].T).astype(ml_dtypes.bfloat16),
                "vaug": va[lo:hi].astype(ml_dtypes.bfloat16),
                "tsm": np.ascontiguousarray(ts[lo:hi].reshape(ntiles, 128).T),
                "maskf": np.ascontiguousarray(mk[lo:hi].reshape(ntiles, 128).T),
            }
        )

    nc = _BUILD_CACHE.get(m_loc)
    if nc is None:
        nc = _build(m_loc)
        _BUILD_CACHE[m_loc] = nc

    res = bass_utils.run_bass_kernel_spmd(
        nc, in_maps, core_ids=list(range(NCORES)), trace=_want_trace
    )

    out = np.empty((B, D), dtype=np.float32)
    for s in range(NCORES):
        out[s * BCHUNK : (s + 1) * BCHUNK] = res.results[s]["out"]
    if _want_trace:
        kernel.last_exec_time_ns = res.exec_time_ns
        kernel.last_results = res
    return out
